# revision 63
# baseline (speedup 1.0000x reference)
"""Multi-head attention TRN2 kernel (b=4, n=2048, e=768, h=8 heads, d=96).

Sharding: 8 cores = 4 batches x 2 head-groups (4 heads each).
Each core computes, for its (batch, head-group):
    qkv projection (its heads' columns of Wqkv), per-head attention
    (softmax over full n=2048), and a partial output projection
    (its heads' rows of Wproj). Host sums the two partial outputs per
    batch (row-parallel linear unshard) and concatenates batches.

All matmul operands are bf16 (PE runs 1 col/cycle for both bf16 and
f32r, but bf16 halves SBUF footprint and input DMA, letting all four
heads' qT/kT stay resident); PSUM accumulation is fp32. Scores are
computed transposed (ET[nk, nq]) so no on-chip transposes are needed;
softmax denominators come from an extra ones-column appended to V
(row 96 of the PV accumulator). exp() skips max-subtraction: logits
are bounded (~|2|) for this problem.

Passes run qh-major ((h0..h3, qh=0) then (h0..h3, qh=1)) so that by the
q1 passes all qh=0 norms are done and the output projection for the
first half of the rows interleaves into the exp-gated bubbles of the
remaining attention passes. Each pass's kb loop is fed a paced queue of
independent PE "fill jobs" (next head's QK projection, deferred
q1-column projections, V projection tail, out-proj blocks) so the PE
never waits on the scalar-engine exp chain.
"""

import os

import ml_dtypes
import numpy as np

import concourse.bacc as bacc
import concourse.mybir as mybir
import concourse.tile as tile
from concourse.bass_utils import run_bass_kernel_spmd

B, N, E = 4, 2048, 768
H = 8          # total heads
HL = 4         # heads per core
D = E // H     # 96
DH = D + 1     # 97 (with denominator column)
KB = E // 128  # 6 contraction blocks
NB = N // 128  # 16 row blocks
NC = 8         # cores
EL = HL * D    # 384 local e-dim
SCALE = float(E) ** -0.5

F32 = mybir.dt.float32
F32R = mybir.dt.float32r
BF16 = mybir.dt.bfloat16
AF = mybir.ActivationFunctionType
MULT = mybir.AluOpType.mult
ADD = mybir.AluOpType.add

_COMPILED = None
LAST_EXEC_NS = None
LAST_RESULTS = None


def _device_reset():
    """Recover a wedged NeuronCore (NRT_EXEC_UNIT_UNRECOVERABLE) via axon."""
    try:
        import ctypes
        import time

        import jax

        jax.devices()
        lib = ctypes.CDLL("/opt/axon/libaxon_pjrt.so")
        lib.axon_reset.restype = ctypes.c_int64
        lib.axon_reset()
        time.sleep(3)
    except Exception:
        pass


def _build():
    nc = bacc.Bacc("TRN2", target_bir_lowering=False, debug=False)

    # weights arrive pre-packed [128 partitions, blocks, cols] so each
    # tensor is ONE dma (descriptor issue cost dominates with many small
    # DMAs); xT is packed the same way but split into 4 column-chunk DMAs
    # so compute can start as chunks land
    xT_d = nc.dram_tensor("xT", [128, KB, N], BF16, kind="ExternalInput")
    wq_d = nc.dram_tensor("wq", [128, KB, EL], BF16, kind="ExternalInput")
    wk_d = nc.dram_tensor("wk", [128, KB, EL], BF16, kind="ExternalInput")
    wv_d = nc.dram_tensor("wv", [128, KB, HL * DH], BF16, kind="ExternalInput")
    bq_d = nc.dram_tensor("bq", [D, HL], F32, kind="ExternalInput")
    bk_d = nc.dram_tensor("bk", [D, HL], F32, kind="ExternalInput")
    bv_d = nc.dram_tensor("bv", [1, HL * DH], F32, kind="ExternalInput")
    wph_d = nc.dram_tensor("wph", [D, HL, E], BF16, kind="ExternalInput")
    bp_d = nc.dram_tensor("bp", [1, E], F32, kind="ExternalInput")
    ones_d = nc.dram_tensor("ones", [1, 128], F32, kind="ExternalInput")
    out_d = nc.dram_tensor("out", [N, E], F32, kind="ExternalOutput")

    with tile.TileContext(nc) as tc:
        with (
            tc.tile_pool(name="const", bufs=1) as cpool,
            tc.tile_pool(name="xt", bufs=1) as xpool,
            tc.tile_pool(name="vh", bufs=1) as vpool,
            tc.tile_pool(name="pt", bufs=4) as ptpool,
            tc.tile_pool(name="nrm", bufs=2) as npool,
            tc.tile_pool(name="pp", bufs=2, space="PSUM") as pp,
            tc.tile_pool(name="pattn", bufs=1, space="PSUM") as pattn,
        ):
            # ---- input DMA. The tiny bias tensors lead the (cheap-issue)
            # scalar queue, all plain f32 — the three one-off broadcast
            # matmuls they feed run quarter-rate, which is irrelevant —
            # so the bvb broadcast matmul (first PE op) runs at ~1us.
            bv_sb = cpool.tile([1, HL * DH], F32, tag="bv")
            nc.scalar.dma_start(bv_sb[:], bv_d[:])
            ones32_sb = cpool.tile([1, 128], F32, tag="ones32")
            nc.scalar.dma_start(ones32_sb[:], ones_d[:])
            bp_sb = cpool.tile([1, E], F32, tag="bp")
            nc.scalar.dma_start(bp_sb[:], bp_d[:])
            # per-kb descriptors spray across DMA engines (one big strided
            # DMA runs ~3x slower); issue cost on the SP queue is trivial
            wq3 = cpool.tile([128, KB, EL], BF16, tag="wq3")
            wk3 = cpool.tile([128, KB, EL], BF16, tag="wk3")
            wv3 = cpool.tile([128, KB, HL * DH], BF16, tag="wv3")
            for kb in range(KB):
                nc.sync.dma_start(wq3[:, kb, :], wq_d[:, kb, :])
            for kb in range(KB):
                nc.sync.dma_start(wk3[:, kb, :], wk_d[:, kb, :])
            for kb in range(KB):
                nc.sync.dma_start(wv3[:, kb, :], wv_d[:, kb, :])
            bq_sb = cpool.tile([D, HL], F32, tag="bq")
            nc.scalar.dma_start(bq_sb[:], bq_d[:])
            bk_sb = cpool.tile([D, HL], F32, tag="bk")
            nc.scalar.dma_start(bk_sb[:], bk_d[:])
            # per-(chunk, kb) descriptors: many outstanding DMAs spray across
            # the DMA engines (a single big strided DMA runs ~3x slower)
            xT3 = xpool.tile([128, KB, N], BF16, tag="xt3")
            for c in range(4):
                eng = [nc.scalar, nc.scalar, nc.sync, nc.sync][c]
                for kb in range(KB):
                    eng.dma_start(
                        xT3[:, kb, c * 512:(c + 1) * 512],
                        xT_d[:, kb, c * 512:(c + 1) * 512],
                    )
            wph3 = cpool.tile([D, HL, E], BF16, tag="wph3")
            nc.sync.dma_start(wph3[:], wph_d[:])

            # broadcast V bias (one K=1 matmul, reused by every vproj block)
            bvb_sb = cpool.tile([128, HL * DH], F32, tag="bvb")
            ps = pp.tile([128, 512], F32, tag="pp")
            nc.tensor.matmul(ps[:, 0:HL * DH], ones32_sb[:], bv_sb[:], start=True, stop=True)
            nc.vector.tensor_copy(bvb_sb[:], ps[:, 0:HL * DH])
            bpb_sb = cpool.tile([128, E], F32, tag="bpb")

            # persistent per-head qT/kT (bf16 halves SBUF: all 8 stay live)
            qT = [
                cpool.tile([D, N], BF16, tag=f"qT{h}", name=f"qT{h}")
                for h in range(HL)
            ]
            kT = [
                cpool.tile([D, N], BF16, tag=f"kT{h}", name=f"kT{h}")
                for h in range(HL)
            ]
            vhat = [
                vpool.tile([128, HL * DH], BF16, tag=f"vh{nb}", name=f"vh{nb}")
                for nb in range(NB)
            ]

            def emit_vproj(nb):
                with nc.named_scope(f"vproj{nb}"):
                    ps = pp.tile([128, 512], F32, tag="pp")
                    for kb in range(KB):
                        nc.tensor.matmul(
                            ps[:, 0:HL * DH],
                            xT3[:, kb, nb * 128:(nb + 1) * 128],
                            wv3[:, kb, :],
                            start=(kb == 0),
                            stop=(kb == KB - 1),
                        )
                    nc.vector.tensor_tensor(vhat[nb][:], ps[:, 0:HL * DH], bvb_sb[:], ADD)

            def emit_qk_chunk(h, i):
                """i in 0..7 -> (q|k, column chunk c)."""
                qk, c = divmod(i, 4)
                w3, b_sb, dst, sc = [
                    (wq3, bq_sb, qT[h], SCALE),
                    (wk3, bk_sb, kT[h], 1.0),
                ][qk]
                with nc.named_scope(f"qkproj{h}"):
                    ps = pp.tile([128, 512], F32, tag="pp", name=f"psqk{h}_{i}")
                    for kb in range(KB):
                        nc.tensor.matmul(
                            ps[0:D, :],
                            w3[:, kb, h * D:(h + 1) * D],
                            xT3[:, kb, c * 512:(c + 1) * 512],
                            start=(kb == 0),
                            stop=(kb == KB - 1),
                        )
                    nc.vector.tensor_scalar(
                        dst[:, c * 512:(c + 1) * 512],
                        ps[0:D, :],
                        sc,
                        b_sb[:, h:h + 1],
                        MULT,
                        ADD,
                    )

            def emit_bpb():
                with nc.named_scope("bpb"):
                    for off, w in [(0, 512), (512, 256)]:
                        ps = pp.tile([128, 512], F32, tag="pp")
                        nc.tensor.matmul(
                            ps[:, 0:w], ones32_sb[:], bp_sb[:, off:off + w],
                            start=True, stop=True,
                        )
                        nc.vector.tensor_copy(bpb_sb[:, off:off + w], ps[:, 0:w])

            # persistent normalized attention outputs (fed straight to the
            # SBUF-direct out-projection; no DRAM staging roundtrip)
            otq = [
                [
                    [
                        cpool.tile(
                            [D, 512], BF16,
                            tag=f"otq{qh}_{h}_{j}", name=f"otq{qh}_{h}_{j}",
                        )
                        for j in range(2)
                    ]
                    for h in range(HL)
                ]
                for qh in range(2)
            ]

            def emit_norm(job, after=(None, None), tail=False):
                h, qh, acc_sb, rec32 = job
                with nc.named_scope(f"norm{h}_{qh}"):
                    for j in range(2):
                        if tail:
                            # latency-critical: PE K=1 broadcast (fp32 is
                            # quarter-rate but skips the f32r cast)
                            bcp = pp.tile([128, 512], F32, tag="pp")
                            nc.tensor.matmul(
                                bcp[0:D, :],
                                ones32_sb[:, 0:D],
                                rec32[:, j * 512:(j + 1) * 512],
                                start=True,
                                stop=True,
                            )
                            bc = bcp[0:D, :]
                        else:
                            # deferred norms: broadcast on gpsimd, zero PE
                            bcs = npool.tile([D, 512], F32, tag="bcs")
                            nc.gpsimd.partition_broadcast(
                                bcs[:], rec32[:, j * 512:(j + 1) * 512]
                            )
                            bc = bcs[:]
                        nc.vector.tensor_tensor(
                            otq[qh][h][j][:],
                            acc_sb[0:D, j * 512:(j + 1) * 512],
                            bc,
                            MULT,
                        )
                        if after[j] is not None:
                            after[j]()

            def emit_out_direct(nb):
                """out-proj from SBUF: 4 per-head K=96 accumulating matmuls
                per column chunk, no DRAM staging roundtrip."""
                qh, r = divmod(nb, 8)
                j, i = divmod(r, 4)
                with nc.named_scope(f"oprojd{nb}"):
                    osb = npool.tile([128, E], F32, tag="osb", bufs=3)
                    for off, w in [(0, 512), (512, 256)]:
                        po = pp.tile([128, 512], F32, tag="pp")
                        for h in range(HL):
                            nc.tensor.matmul(
                                po[:, 0:w],
                                otq[qh][h][j][:, i * 128:(i + 1) * 128],
                                wph3[:, h, off:off + w],
                                start=(h == 0),
                                stop=(h == HL - 1),
                            )
                        nc.vector.tensor_tensor(
                            osb[:, off:off + w], po[:, 0:w], bpb_sb[:, off:off + w], ADD
                        )
                    nc.gpsimd.dma_start(out_d[nb * 128:(nb + 1) * 128, :], osb[:])

            # ---- pre-phase: h0 critical qk chunks interleaved with the V
            # projection, following the xT chunk DMA arrival order ----
            # critical = q chunks for qh0 (c0,c1) + all k chunks; q c2/c3 are
            # only needed by the qh=1 pass and are deferred as fill jobs.
            CRIT = [0, 4, 1, 5, 6, 7]   # (q,c0),(k,c0),(q,c1),(k,c1),(k,c2),(k,c3)
            DEFER = [2, 3]              # (q,c2),(q,c3)
            pre = [
                (0, 0), "bpb", (0, 4), (0, 1), (0, 5),
                "v0", "v1", "v2", "v3", (0, 6), (0, 7), "v4", "v5",
            ]
            for item in pre:
                if item == "bpb":
                    emit_bpb()
                elif isinstance(item, str):
                    emit_vproj(int(item[1:]))
                else:
                    emit_qk_chunk(*item)

            # fill-job queues per pass: (min_kb, closure)
            def J(fn, *a, min_kb=0):
                return (min_kb, lambda: fn(*a))

            fills = {
                (0, 0): [J(emit_vproj, nb) for nb in range(6, NB)]
                        + [J(emit_qk_chunk, 1, i) for i in CRIT],
                (1, 0): [J(emit_qk_chunk, 2, i) for i in CRIT]
                        + [J(emit_qk_chunk, 0, i) for i in DEFER],
                (2, 0): [J(emit_qk_chunk, 3, i) for i in CRIT]
                        + [J(emit_qk_chunk, 1, i) for i in DEFER],
                (3, 0): [J(emit_qk_chunk, 2, i) for i in DEFER]
                        + [J(emit_qk_chunk, 3, i) for i in DEFER],
                # q1 passes: out-proj for the qh=0 half interleaves in.
                # nb0-7 need every head's qh=0 norm; the last one (h3,q0) is
                # emitted at kb==2 of pass (0,1), so gate on kb>=4 there.
                (0, 1): [J(emit_out_direct, nb, min_kb=4) for nb in range(0, 2)],
                (1, 1): [J(emit_out_direct, nb) for nb in range(2, 4)],
                (2, 1): [J(emit_out_direct, nb) for nb in range(4, 6)],
                (3, 1): [J(emit_out_direct, nb) for nb in range(6, 8)],
            }

            pending = None
            for qh in range(2):
                for h in range(HL):
                    jobs = fills[(h, qh)]
                    popped = 0
                    with nc.named_scope(f"attn{h}_{qh}"):
                        acc = pattn.tile([DH, 1024], F32, tag="acc")

                        def emit_pv(kbp, pt):
                            for j in range(2):
                                nc.tensor.matmul(
                                    acc[:, j * 512:(j + 1) * 512],
                                    vhat[kbp][:, h * DH:(h + 1) * DH],
                                    pt[:, j * 512:(j + 1) * 512],
                                    start=(kbp == 0),
                                    stop=(kbp == NB - 1),
                                )

                        pvq = []
                        for kb in range(NB):
                            et = pattn.tile([128, 1024], F32, tag="et", bufs=2)
                            for j in range(2):
                                c = 2 * qh + j
                                nc.tensor.matmul(
                                    et[:, j * 512:(j + 1) * 512],
                                    kT[h][:, kb * 128:(kb + 1) * 128],
                                    qT[h][:, c * 512:(c + 1) * 512],
                                    start=True,
                                    stop=True,
                                )
                            # PV runs two steps behind exp so the PE never
                            # waits on the scalar-engine exp latency
                            if len(pvq) == 2:
                                emit_pv(*pvq.pop(0))
                            pt = ptpool.tile([128, 1024], BF16, tag="pt")
                            nc.scalar.activation(pt[:], et[:], AF.Exp)
                            pvq.append((kb, pt))
                            if kb == 2 and pending is not None:
                                emit_norm(pending)
                                pending = None
                            # paced fill: spread jobs evenly over eligible kbs
                            target = (kb + 1) * len(jobs) // NB
                            while popped < len(jobs) and popped < target \
                                    and jobs[popped][0] <= kb:
                                jobs[popped][1]()
                                popped += 1
                        while popped < len(jobs):
                            jobs[popped][1]()
                            popped += 1
                        for job in pvq:
                            emit_pv(*job)
                        # reciprocal chain now (off the next pass / tail
                        # critical path); sums staged through SBUF (recip
                        # can't read PSUM directly), j-halved so the tail's
                        # first broadcast isn't gated on the full row
                        sums = npool.tile([1, 1024], F32, tag="sums", bufs=1)
                        rec32 = npool.tile([1, 1024], F32, tag="rec32")
                        for j in range(2):
                            sl = slice(j * 512, (j + 1) * 512)
                            nc.vector.tensor_copy(sums[:, sl], acc[D:DH, sl])
                            nc.vector.reciprocal_approx_fast(rec32[:, sl], sums[:, sl])
                        acc_sb = npool.tile([DH, 1024], F32, tag="acc_sb")
                        nc.vector.tensor_copy(acc_sb[:, 0:512], acc[:, 0:512])
                        nc.scalar.copy(acc_sb[:, 512:1024], acc[:, 512:1024])
                        pending = (h, qh, acc_sb, rec32)

            # ---- tail: last norm overlapped with SBUF-direct out-proj ----
            emit_norm(
                pending,
                after=(
                    lambda: [emit_out_direct(nb) for nb in range(8, 12)],
                    lambda: [emit_out_direct(nb) for nb in range(12, NB)],
                ),
                tail=True,
            )

    nc.compile()
    return nc


def _pack(a, blocks):
    """[blocks*128, cols] -> [128, blocks, cols] (partition-major packing)."""
    cols = a.shape[1]
    return np.ascontiguousarray(a.reshape(blocks, 128, cols).transpose(1, 0, 2))


def _shard(x, Wqkv, bqkv, Wproj, bproj):
    """Build per-core input maps. Core c -> (batch c//2, head-group c%2)."""
    BF = ml_dtypes.bfloat16
    Wr = np.ascontiguousarray(Wqkv.reshape(E, H, D, 3))
    br = np.ascontiguousarray(bqkv.reshape(H, D, 3))
    ones = np.ones((1, 128), np.float32)
    in_maps = []
    for c in range(NC):
        bb, hg = divmod(c, 2)
        hs = slice(hg * HL, (hg + 1) * HL)
        wq = _pack(Wr[:, hs, :, 0].reshape(E, EL).astype(BF), KB)
        wk = _pack(Wr[:, hs, :, 1].reshape(E, EL).astype(BF), KB)
        wv = np.zeros((E, HL, DH), np.float32)
        wv[:, :, :D] = Wr[:, hs, :, 2]
        bq = np.ascontiguousarray((br[hs, :, 0] * SCALE).T)  # [D, HL], pre-scaled
        bk = np.ascontiguousarray(br[hs, :, 1].T)
        bv = np.zeros((HL, DH), np.float32)
        bv[:, :D] = br[hs, :, 2]
        bv[:, D] = 1.0  # denominator ones column
        wph = np.ascontiguousarray(
            Wproj[hg * EL:(hg + 1) * EL, :].reshape(HL, D, E)
            .transpose(1, 0, 2).astype(BF)
        )
        bp = bproj if hg == 0 else np.zeros_like(bproj)
        in_maps.append({
            "xT": _pack(x[bb].T.astype(BF), KB),
            "wq": wq,
            "wk": wk,
            "wv": _pack(wv.reshape(E, HL * DH).astype(BF), KB),
            "bq": bq,
            "bk": bk,
            "bv": np.ascontiguousarray(bv.reshape(1, HL * DH)),
            "wph": wph,
            "bp": np.ascontiguousarray(bp.reshape(1, E)),
            "ones": ones,
        })
    return in_maps


def kernel(x, Wqkv, bqkv, Wproj, bproj):
    global _COMPILED, LAST_EXEC_NS, LAST_RESULTS
    x = np.asarray(x, dtype=np.float32)
    Wqkv = np.asarray(Wqkv, dtype=np.float32)
    bqkv = np.asarray(bqkv, dtype=np.float32)
    Wproj = np.asarray(Wproj, dtype=np.float32)
    bproj = np.asarray(bproj, dtype=np.float32)

    if _COMPILED is None:
        _COMPILED = _build()
    nc = _COMPILED

    in_maps = _shard(x, Wqkv, bqkv, Wproj, bproj)
    trace = bool(int(os.environ.get("BASS_MHA_TRACE", "0")))
    try:
        res = run_bass_kernel_spmd(nc, in_maps, list(range(NC)), trace=trace)
    except Exception:
        _device_reset()
        res = run_bass_kernel_spmd(nc, in_maps, list(range(NC)), trace=trace)
    LAST_EXEC_NS = res.exec_time_ns
    LAST_RESULTS = res

    out = np.empty((B, N, E), np.float32)
    for bb in range(B):
        out[bb] = res.results[2 * bb]["out"] + res.results[2 * bb + 1]["out"]
    return out
